# revision 1
# baseline (speedup 1.0000x reference)
"""Causal self-attention (B=2, S=2048, E=2048, H=16) on 8 TRN2 NeuronCores.

Sharding: 2-way batch x 4-way head-group tensor parallel.
Core c handles batch c//4 and heads [4*(c%4), 4*(c%4)+4).

Per-core kernel:
  phase 1: X^T via PE transposes; X arrives host-split into bf16 hi/lo
  phase 2: QKV projection as 3 exact bf16 products (hi*hi + hi*lo + lo*hi,
           ~fp32 precision at 3 cyc/row instead of fp32's 4 cyc/row)
           -> feature-major qT/kT/vT [128hd, S] fp32, staged through DRAM
  phase 3: per head: causal attention (fp32 q-major scores, chunked exp with
           accumulated row-sums, P normalized then PE-transposed, PV)
  phase 4: out projection, attT chip-split to bf16 hi/lo, W_out host-split

Host side: shard + bf16-split inputs, run SPMD on 8 cores, sum the 4
head-group partials per batch and add (b_out + b_v @ W_out) once.
"""

from contextlib import ExitStack

import ml_dtypes
import numpy as np

import concourse.bass as bass
import concourse.tile as tile
from concourse import bacc, bass_utils, mybir
from concourse.masks import make_causal_mask, make_identity

FP = mybir.dt.float32
BF = mybir.dt.bfloat16
AF = mybir.ActivationFunctionType

B = 2
S = 2048
E = 2048
H = 16
HD = 128
NCORES = 8
HG = 4  # head-group axis (tensor parallel)
H_LOC = H // HG  # 4 heads per core
FLOC = H_LOC * HD  # 512 local features per q/k/v
SCALE = 1.0 / float(np.sqrt(HD))
NEG = -1.0e30

PROFILE = False
LAST_EXEC_NS = None
LAST_RESULTS = None


def _emit(nc, S=S, E=E):
    NB = S // 128
    EB = E // 128
    xh = nc.dram_tensor("xh", [S, E], BF, kind="ExternalInput").ap()
    xl = nc.dram_tensor("xl", [S, E], BF, kind="ExternalInput").ap()
    wqkv_hl = []
    for wn in ("wq", "wk", "wv"):
        pair = []
        for p in ("h", "l"):
            pair.append(
                nc.dram_tensor(f"{wn}{p}", [E, FLOC], BF, kind="ExternalInput").ap()
            )
        wqkv_hl.append(pair)
    bqs = nc.dram_tensor("bqs", [FLOC, 1], FP, kind="ExternalInput").ap()  # *SCALE
    bk = nc.dram_tensor("bk", [FLOC, 1], FP, kind="ExternalInput").ap()
    woh = nc.dram_tensor("woh", [FLOC, E], BF, kind="ExternalInput").ap()
    wol = nc.dram_tensor("wol", [FLOC, E], BF, kind="ExternalInput").ap()
    out = nc.dram_tensor("out", [S, E], FP, kind="ExternalOutput").ap()

    with tile.TileContext(nc) as tc, ExitStack() as top:
        dram = top.enter_context(tc.tile_pool(name="dram", bufs=1, space="DRAM"))
        # feature-major fp32 scratch: per head 128 rows (hd) x S cols
        qT = [dram.tile([128, S], FP, name=f"qT{h}", tag=f"qT{h}") for h in range(H_LOC)]
        kT = [dram.tile([128, S], FP, name=f"kT{h}", tag=f"kT{h}") for h in range(H_LOC)]
        vT = [dram.tile([128, S], FP, name=f"vT{h}", tag=f"vT{h}") for h in range(H_LOC)]
        qkvT = [qT, kT, vT]

        cst = top.enter_context(tc.tile_pool(name="cst", bufs=1))
        ident = cst.tile([128, 128], FP, name="ident", tag="ident")
        make_identity(nc, ident[:])
        ident_bf = cst.tile([128, 128], BF, name="identbf", tag="identbf")
        make_identity(nc, ident_bf[:])
        cmask = cst.tile([128, 128], FP, name="cmask", tag="cmask")
        make_causal_mask(nc, cmask[:], mask_val=NEG)
        bq_sb = cst.tile([128, H_LOC], FP, name="bq", tag="bq")
        bk_sb = cst.tile([128, H_LOC], FP, name="bk", tag="bk")
        for f in range(H_LOC):
            nc.sync.dma_start(bq_sb[:, f : f + 1], bqs[128 * f : 128 * (f + 1), :])
            nc.sync.dma_start(bk_sb[:, f : f + 1], bk[128 * f : 128 * (f + 1), :])

        ps_aux = top.enter_context(tc.tile_pool(name="ps_aux", bufs=4, space="PSUM"))

        # ---------------- phase 1+2: X^T and QKV projection ----------------
        with ExitStack() as ph, nc.named_scope("proj"):
            xt_pool = ph.enter_context(tc.tile_pool(name="xt", bufs=1))
            xts = [
                [
                    xt_pool.tile([128, S], BF, name=f"xt{p}{j}", tag=f"xt{p}{j}")
                    for j in range(EB)
                ]
                for p in range(2)  # 0=hi, 1=lo
            ]
            xin = ph.enter_context(tc.tile_pool(name="xin", bufs=4))
            ps_main = ph.enter_context(
                tc.tile_pool(name="ps_main", bufs=4, space="PSUM")
            )
            wpool = ph.enter_context(tc.tile_pool(name="w", bufs=4))
            stg = ph.enter_context(tc.tile_pool(name="stg", bufs=4))

            # X^T for hi and lo parts
            for p, xsrc in enumerate((xh, xl)):
                for ig in range(NB // 4):
                    xrow = []
                    for m in range(4):
                        i = 4 * ig + m
                        xr = xin.tile([128, E], BF, name="xin", tag="xin")
                        nc.sync.dma_start(xr[:], xsrc[128 * i : 128 * (i + 1), :])
                        xrow.append(xr)
                    for j in range(EB):
                        pt = ps_main.tile([128, 512], BF, name="psb", tag="ps")
                        for m in range(4):
                            nc.tensor.transpose(
                                pt[:, 128 * m : 128 * (m + 1)],
                                xrow[m][:, 128 * j : 128 * (j + 1)],
                                ident_bf[:],
                            )
                        nc.scalar.activation(
                            xts[p][j][:, 512 * ig : 512 * (ig + 1)], pt[:], AF.Copy
                        )

            # projections: 3 bf16 products, f-block == (which, head)
            for which in range(3):  # q, k, v
                wth_d, wtl_d = wqkv_hl[which]
                for h in range(H_LOC):
                    nsc = S // 512
                    psums = []
                    for sc in range(nsc):
                        psums.append(ps_main.tile([128, 512], FP, name="ps", tag="ps"))
                    for e in range(EB):
                        wth = wpool.tile([128, 128], BF, name="wh", tag="wh")
                        nc.sync.dma_start(
                            wth[:],
                            wth_d[128 * e : 128 * (e + 1), 128 * h : 128 * (h + 1)],
                        )
                        wtl = wpool.tile([128, 128], BF, name="wl", tag="wl")
                        nc.sync.dma_start(
                            wtl[:],
                            wtl_d[128 * e : 128 * (e + 1), 128 * h : 128 * (h + 1)],
                        )
                        first = e == 0
                        last = e == EB - 1
                        for sc in range(nsc):
                            sl = slice(512 * sc, 512 * (sc + 1))
                            nc.tensor.matmul(
                                psums[sc][:], wth[:], xts[0][e][:, sl],
                                start=first, stop=False,
                            )
                            nc.tensor.matmul(
                                psums[sc][:], wth[:], xts[1][e][:, sl],
                                start=False, stop=False,
                            )
                            nc.tensor.matmul(
                                psums[sc][:], wtl[:], xts[0][e][:, sl],
                                start=False, stop=last,
                            )
                    for sc in range(nsc):
                        st = stg.tile([128, 512], FP, name="stg", tag="stg")
                        if which == 0:
                            nc.vector.tensor_scalar(
                                st[:], psums[sc][:], SCALE, bq_sb[:, h : h + 1],
                                op0=mybir.AluOpType.mult, op1=mybir.AluOpType.add,
                            )
                        elif which == 1:
                            nc.vector.tensor_scalar_add(
                                st[:], psums[sc][:], bk_sb[:, h : h + 1]
                            )
                        else:
                            nc.scalar.activation(st[:], psums[sc][:], AF.Copy)
                        nc.sync.dma_start(
                            qkvT[which][h][:, 512 * sc : 512 * (sc + 1)], st[:]
                        )

        # ---------------- phase 3: attention per head ----------------
        with ExitStack() as ao:
            att_pool = ao.enter_context(tc.tile_pool(name="att", bufs=1))
            attT = []  # (hi, lo) bf16 pairs
            with ExitStack() as ph:
                qkv_pool = ph.enter_context(tc.tile_pool(name="qkv", bufs=2))
                vsb_pool = ph.enter_context(tc.tile_pool(name="vsb", bufs=2))
                p_pool = ph.enter_context(tc.tile_pool(name="p", bufs=2))
                pt_pool = ph.enter_context(tc.tile_pool(name="pt", bufs=2))
                rs_pool = ph.enter_context(tc.tile_pool(name="rs", bufs=4))
                ps_sc = ph.enter_context(
                    tc.tile_pool(name="ps_sc", bufs=4, space="PSUM")
                )

                for h in range(H_LOC):
                    with nc.named_scope(f"attn{h}"):
                        qt = qkv_pool.tile([128, S], FP, name="qt", tag="qt")
                        kt = qkv_pool.tile([128, S], FP, name="kt", tag="kt")
                        vt = qkv_pool.tile([128, S], FP, name="vt", tag="vt")
                        nc.sync.dma_start(qt[:], qT[h][:])
                        nc.sync.dma_start(kt[:], kT[h][:])
                        nc.sync.dma_start(vt[:], vT[h][:])

                        # V -> token-major [s-block, hd] tiles
                        vsb = vsb_pool.tile([128, S], FP, name="vsb", tag="vsb")
                        for mg in range(NB // 4):
                            pv = ps_aux.tile([128, 512], FP, name="psa", tag="psa")
                            for m in range(4):
                                nc.tensor.transpose(
                                    pv[:, 128 * m : 128 * (m + 1)],
                                    vt[:, 128 * (4 * mg + m) : 128 * (4 * mg + m + 1)],
                                    ident[:],
                                )
                            nc.scalar.activation(
                                vsb[:, 512 * mg : 512 * (mg + 1)], pv[:], AF.Copy
                            )

                        att_h = att_pool.tile(
                            [128, S], BF, name=f"atth{h}", tag=f"atth{h}"
                        )
                        att_l = att_pool.tile(
                            [128, S], BF, name=f"attl{h}", tag=f"attl{h}"
                        )
                        attT.append((att_h, att_l))

                        for g in range(S // 512):  # q-groups of 512
                            PT = pt_pool.tile([128, 4 * S], FP, name="PT", tag="PT")
                            nkc = 4 * (g + 1)  # key chunks of 128 for this group
                            for qs in range(4):
                                i = 4 * g + qs  # q-block
                                L = 128 * (i + 1)
                                nq0 = 128 * i
                                p = p_pool.tile([128, 2048], FP, name="p", tag="p")
                                rs = rs_pool.tile([128, 6], FP, name="rs", tag="rs")
                                ncchunks = (L + 511) // 512
                                for c in range(ncchunks):
                                    w = min(512, L - 512 * c)
                                    psc = ps_sc.tile(
                                        [128, 512], FP, name="psc", tag="psc"
                                    )
                                    nc.tensor.matmul(
                                        psc[:, :w],
                                        qt[:, nq0 : nq0 + 128],
                                        kt[:, 512 * c : 512 * c + w],
                                        start=True,
                                        stop=True,
                                    )
                                    if c == ncchunks - 1:
                                        # causal mask on the diagonal 128 cols
                                        nc.vector.tensor_add(
                                            psc[:, w - 128 : w],
                                            psc[:, w - 128 : w],
                                            cmask[:],
                                        )
                                    nc.scalar.activation(
                                        p[:, 512 * c : 512 * c + w], psc[:, :w],
                                        AF.Exp, accum_out=rs[:, c : c + 1],
                                    )
                                for c in range(1, ncchunks):
                                    nc.vector.tensor_add(
                                        rs[:, 0:1], rs[:, 0:1], rs[:, c : c + 1]
                                    )
                                nc.vector.reciprocal(rs[:, 4:5], rs[:, 0:1])
                                nc.vector.tensor_scalar_mul(
                                    p[:, :L], p[:, :L], rs[:, 4:5]
                                )
                                # transpose P into PT (k-major)
                                for jg in range((i + 1 + 3) // 4):
                                    nm = min(4, i + 1 - 4 * jg)
                                    ptp = ps_aux.tile(
                                        [128, 512], FP, name="psa", tag="psa"
                                    )
                                    for m in range(nm):
                                        j = 4 * jg + m
                                        nc.tensor.transpose(
                                            ptp[:, 128 * m : 128 * (m + 1)],
                                            p[:, 128 * j : 128 * (j + 1)],
                                            ident[:],
                                        )
                                    src = ptp[:, : 128 * nm].rearrange(
                                        "p (m q) -> p m q", q=128
                                    )
                                    dst = PT.rearrange("p (j q) -> p j q", q=512)[
                                        :, 4 * jg : 4 * jg + nm,
                                        128 * qs : 128 * (qs + 1),
                                    ]
                                    nc.scalar.activation(dst, src, AF.Copy)
                            # PV for the group
                            po = ps_aux.tile([128, 512], FP, name="psa", tag="psa")
                            for j in range(nkc):
                                qlo = max(0, 128 * (j - 4 * g))  # causal: q >= k
                                nc.tensor.matmul(
                                    po[:, qlo:512],
                                    vsb[:, 128 * j : 128 * (j + 1)],
                                    PT[:, 512 * j + qlo : 512 * j + 512],
                                    start=(j == 0),
                                    stop=(j == nkc - 1),
                                )
                            gsl = slice(512 * g, 512 * (g + 1))
                            nc.scalar.activation(att_h[:, gsl], po[:], AF.Copy)
                            nc.vector.tensor_sub(att_l[:, gsl], po[:], att_h[:, gsl])

            # ---------------- phase 4: output projection ----------------
            with ExitStack() as ph, nc.named_scope("outproj"):
                wo_pool = ph.enter_context(tc.tile_pool(name="wo", bufs=1))
                ostg = ph.enter_context(tc.tile_pool(name="ostg", bufs=4))
                ps_out = ph.enter_context(
                    tc.tile_pool(name="ps_out", bufs=4, space="PSUM")
                )
                wohs, wols = [], []
                for h in range(H_LOC):
                    wt = wo_pool.tile([128, E], BF, name=f"woh{h}", tag=f"woh{h}")
                    nc.sync.dma_start(wt[:], woh[128 * h : 128 * (h + 1), :])
                    wohs.append(wt)
                    wt = wo_pool.tile([128, E], BF, name=f"wol{h}", tag=f"wol{h}")
                    nc.sync.dma_start(wt[:], wol[128 * h : 128 * (h + 1), :])
                    wols.append(wt)
                nec = E // 512
                for i in range(NB):
                    psums = []
                    for c in range(nec):
                        psums.append(
                            ps_out.tile([128, 512], FP, name="pso", tag="pso")
                        )
                    for h in range(H_LOC):
                        ah = attT[h][0][:, 128 * i : 128 * (i + 1)]
                        al = attT[h][1][:, 128 * i : 128 * (i + 1)]
                        first = h == 0
                        last = h == H_LOC - 1
                        for c in range(nec):
                            sl = slice(512 * c, 512 * (c + 1))
                            nc.tensor.matmul(
                                psums[c][:], ah, wohs[h][:, sl],
                                start=first, stop=False,
                            )
                            nc.tensor.matmul(
                                psums[c][:], ah, wols[h][:, sl],
                                start=False, stop=False,
                            )
                            nc.tensor.matmul(
                                psums[c][:], al, wohs[h][:, sl],
                                start=False, stop=last,
                            )
                    for c in range(nec):
                        ot = ostg.tile([128, 512], FP, name="ostg", tag="ostg")
                        nc.scalar.activation(ot[:], psums[c][:], AF.Copy)
                        nc.sync.dma_start(
                            out[128 * i : 128 * (i + 1), 512 * c : 512 * (c + 1)],
                            ot[:],
                        )


_NC_CACHE = None


def _get_nc():
    global _NC_CACHE
    if _NC_CACHE is None:
        nc = bacc.Bacc(
            "TRN2",
            target_bir_lowering=False,
            debug=False,
            num_devices=1,
            enable_asserts=False,
        )
        _emit(nc)
        nc.compile()
        _NC_CACHE = nc
    return _NC_CACHE


def _split(a):
    hi = a.astype(ml_dtypes.bfloat16)
    lo = (a - hi.astype(np.float32)).astype(ml_dtypes.bfloat16)
    return hi, lo


def make_in_maps(inX, W_qkv, b_qkv, W_out):
    in_maps = []
    for c in range(NCORES):
        b = c // HG
        hg = c % HG
        sl = slice(FLOC * hg, FLOC * (hg + 1))
        xh_, xl_ = _split(inX[b])
        wqh_, wql_ = _split(W_qkv[:, 0:E][:, sl])
        wkh_, wkl_ = _split(W_qkv[:, E : 2 * E][:, sl])
        wvh_, wvl_ = _split(W_qkv[:, 2 * E : 3 * E][:, sl])
        woh_, wol_ = _split(W_out[sl, :])
        in_maps.append(
            {
                "xh": np.ascontiguousarray(xh_),
                "xl": np.ascontiguousarray(xl_),
                "wqh": np.ascontiguousarray(wqh_),
                "wql": np.ascontiguousarray(wql_),
                "wkh": np.ascontiguousarray(wkh_),
                "wkl": np.ascontiguousarray(wkl_),
                "wvh": np.ascontiguousarray(wvh_),
                "wvl": np.ascontiguousarray(wvl_),
                "bqs": np.ascontiguousarray(
                    (b_qkv[0:E][sl] * SCALE).reshape(FLOC, 1)
                ),
                "bk": np.ascontiguousarray(b_qkv[E : 2 * E][sl].reshape(FLOC, 1)),
                "woh": np.ascontiguousarray(woh_),
                "wol": np.ascontiguousarray(wol_),
            }
        )
    return in_maps


def kernel(inX, W_qkv, b_qkv, W_out, b_out):
    global LAST_EXEC_NS, LAST_RESULTS
    inX = np.asarray(inX, dtype=np.float32)
    W_qkv = np.asarray(W_qkv, dtype=np.float32)
    b_qkv = np.asarray(b_qkv, dtype=np.float32)
    W_out = np.asarray(W_out, dtype=np.float32)
    b_out = np.asarray(b_out, dtype=np.float32)

    nc = _get_nc()
    in_maps = make_in_maps(inX, W_qkv, b_qkv, W_out)

    kwargs = {}
    if PROFILE:
        kwargs = {"trace": True, "trace_cores": [0]}
    res = bass_utils.run_bass_kernel_spmd(
        nc, in_maps, core_ids=list(range(NCORES)), **kwargs
    )
    LAST_EXEC_NS = res.exec_time_ns
    LAST_RESULTS = res

    bias_full = (b_out + b_qkv[2 * E : 3 * E] @ W_out).astype(np.float32)
    out = np.empty((B, S, E), dtype=np.float32)
    for b in range(B):
        acc = res.results[HG * b + 0]["out"].astype(np.float64)
        for hg in range(1, HG):
            acc += res.results[HG * b + hg]["out"]
        out[b] = (acc + bias_full).astype(np.float32)
    return out



# revision 8
# speedup vs baseline: 2.1089x; 2.1089x over previous
"""Causal self-attention (B=2, S=2048, E=2048, H=16) on 8 TRN2 NeuronCores.

Sharding: 2-way batch x 4-way head-group tensor parallel.
Core c handles batch c//4 and heads [4*(c%4), 4*(c%4)+4).

Single-pass bf16 design (the rel-err gate is 2e-2; bf16 lands ~3e-3):
  phase 1: QKV projection from host-pretransposed X^T (bf16), feature-major
           qT/kT/vT [128hd, S] bf16 kept in SBUF (no DRAM staging).
           SCALE is folded into Wq/bq on the host.
  phase 2: per head: k-major scores (stationary kT block, moving qT) ->
           transposed-causal mask -> exp to bf16 expPT (k-major, which is
           exactly the PV moving layout: no P transposes). Row sums via a
           ones-column matmul accumulated alongside PV; normalization is
           applied to the (small) PV output, with the per-q reciprocal
           broadcast across partitions by a tiny K=1 matmul.
  phase 3: out projection: stationary attO blocks (feature-major), moving
           W_out rows; fp32 partials to DRAM.

Host side: per batch X^T in bf16 (shared by 4 cores), per head-group W
slices in bf16; sum the 4 head-group partials per batch and add
(b_out + b_v @ W_out) once (softmax rows sum to 1, so the v-bias
contribution is a constant row vector).
"""

from contextlib import ExitStack

import ml_dtypes
import numpy as np

import concourse.bass as bass
import concourse.tile as tile
from concourse import bacc, bass_utils, mybir
from concourse.masks import make_identity

FP = mybir.dt.float32
BF = mybir.dt.bfloat16
AF = mybir.ActivationFunctionType

B = 2
S = 2048
E = 2048
H = 16
HD = 128
NCORES = 8
HG = 4  # head-group axis (tensor parallel)
H_LOC = H // HG  # 4 heads per core
FLOC = H_LOC * HD  # 512 local features per q/k/v
SCALE = 1.0 / float(np.sqrt(HD))
NEG = -1.0e30

PROFILE = False
LAST_EXEC_NS = None
LAST_RESULTS = None


def _emit(nc, S=S, E=E):
    NB = S // 128  # token blocks
    EB = E // 128  # embed blocks
    NC = S // 512  # 512-wide q chunks

    xT = nc.dram_tensor("xT", [E, S], BF, kind="ExternalInput").ap()
    wq = nc.dram_tensor("wq", [E, FLOC], BF, kind="ExternalInput").ap()
    wk = nc.dram_tensor("wk", [E, FLOC], BF, kind="ExternalInput").ap()
    wv = nc.dram_tensor("wv", [E, FLOC], BF, kind="ExternalInput").ap()
    bqs = nc.dram_tensor("bqs", [FLOC, 1], FP, kind="ExternalInput").ap()  # *SCALE
    bk = nc.dram_tensor("bk", [FLOC, 1], FP, kind="ExternalInput").ap()
    wo = nc.dram_tensor("wo", [FLOC, E], BF, kind="ExternalInput").ap()
    out = nc.dram_tensor("out", [S, E], FP, kind="ExternalOutput").ap()

    with tile.TileContext(nc) as tc, ExitStack() as top:
        cst = top.enter_context(tc.tile_pool(name="cst", bufs=1))
        ident_bf = cst.tile([128, 128], BF, name="identbf", tag="identbf")
        make_identity(nc, ident_bf[:])
        # transposed causal mask: keep (0) where q(free) >= k(part), NEG below
        maskT = cst.tile([128, 128], FP, name="maskT", tag="maskT")
        nc.gpsimd.memset(maskT[:], 0.0)
        nc.gpsimd.affine_select(
            out=maskT[:],
            in_=maskT[:],
            compare_op=mybir.AluOpType.is_ge,
            fill=NEG,
            base=0,
            pattern=[[1, 128]],  # +1 * free index
            channel_multiplier=-1,  # -1 * partition index
        )
        ones_col = cst.tile([128, 1], BF, name="onescol", tag="onescol")
        nc.vector.memset(ones_col[:], 1.0)
        ones_row = cst.tile([1, 128], FP, name="onesrow", tag="onesrow")
        nc.vector.memset(ones_row[:], 1.0)
        bq_sb = cst.tile([128, H_LOC], FP, name="bq", tag="bq")
        bk_sb = cst.tile([128, H_LOC], FP, name="bk", tag="bk")
        for f in range(H_LOC):
            nc.sync.dma_start(bq_sb[:, f : f + 1], bqs[128 * f : 128 * (f + 1), :])
            nc.sync.dma_start(bk_sb[:, f : f + 1], bk[128 * f : 128 * (f + 1), :])

        # feature-major q/k/v, bf16, SBUF-resident
        qkv_pool = top.enter_context(tc.tile_pool(name="qkvT", bufs=1))
        qT = [qkv_pool.tile([128, S], BF, name=f"qT{h}", tag=f"qT{h}") for h in range(H_LOC)]
        kT = [qkv_pool.tile([128, S], BF, name=f"kT{h}", tag=f"kT{h}") for h in range(H_LOC)]
        vT = [qkv_pool.tile([128, S], BF, name=f"vT{h}", tag=f"vT{h}") for h in range(H_LOC)]
        qkvT = [qT, kT, vT]
        # feature-major attention outputs, bf16, SBUF-resident
        att_pool = top.enter_context(tc.tile_pool(name="att", bufs=1))
        attO = [
            att_pool.tile([128, S], BF, name=f"attO{h}", tag=f"attO{h}")
            for h in range(H_LOC)
        ]

        # ---------------- phase 1: QKV projection ----------------
        with ExitStack() as ph, nc.named_scope("proj"):
            xt_pool = ph.enter_context(tc.tile_pool(name="xt", bufs=1))
            xts = []
            for e in range(EB):
                t = xt_pool.tile([128, S], BF, name=f"xt{e}", tag=f"xt{e}")
                nc.sync.dma_start(t[:], xT[128 * e : 128 * (e + 1), :])
                xts.append(t)
            wpool = ph.enter_context(tc.tile_pool(name="w", bufs=4))
            ps_qkv = ph.enter_context(tc.tile_pool(name="ps_qkv", bufs=8, space="PSUM"))

            for which, wsrc in enumerate((wq, wk, wv)):
                for h in range(H_LOC):
                    psums = [
                        ps_qkv.tile([128, 512], FP, name="psq", tag="psq")
                        for _ in range(NC)
                    ]
                    for e in range(EB):
                        wt = wpool.tile([128, 128], BF, name="wt", tag="wt")
                        nc.sync.dma_start(
                            wt[:],
                            wsrc[128 * e : 128 * (e + 1), 128 * h : 128 * (h + 1)],
                        )
                        first = e == 0
                        last = e == EB - 1
                        for sc in range(NC):
                            nc.tensor.matmul(
                                psums[sc][:],
                                wt[:],
                                xts[e][:, 512 * sc : 512 * (sc + 1)],
                                start=first,
                                stop=last,
                            )
                    dst = qkvT[which][h]
                    for sc in range(NC):
                        sl = slice(512 * sc, 512 * (sc + 1))
                        if which == 0:
                            nc.vector.tensor_scalar_add(
                                dst[:, sl], psums[sc][:], bq_sb[:, h : h + 1]
                            )
                        elif which == 1:
                            nc.vector.tensor_scalar_add(
                                dst[:, sl], psums[sc][:], bk_sb[:, h : h + 1]
                            )
                        else:
                            nc.scalar.activation(dst[:, sl], psums[sc][:], AF.Copy)

        # ---------------- phase 2: attention per head ----------------
        with ExitStack() as ah:
            vsb_pool = ah.enter_context(tc.tile_pool(name="vsb", bufs=2))
            ept_pool = ah.enter_context(tc.tile_pool(name="ept", bufs=2))
            rsb_pool = ah.enter_context(tc.tile_pool(name="rsb", bufs=2))
            bcs_pool = ah.enter_context(tc.tile_pool(name="bcs", bufs=2))
            ps_sc = ah.enter_context(tc.tile_pool(name="ps_sc", bufs=2, space="PSUM"))
            ps_pv = ah.enter_context(tc.tile_pool(name="ps_pv", bufs=4, space="PSUM"))
            ps_rs = ah.enter_context(tc.tile_pool(name="ps_rs", bufs=2, space="PSUM"))
            # PE matmul outputs must start at partition 0/32/64: spread the
            # 4 per-chunk rowsum rows over two banks at those offsets
            RS_POS = [(0, 0), (0, 32), (0, 64), (1, 0)]

            for h in range(H_LOC):
                with nc.named_scope(f"attn{h}"):
                    # V -> token-major [128 tok, hd] blocks along S
                    vsb = vsb_pool.tile([128, S], BF, name="vsb", tag="vsb")
                    for mg in range(NB // 4):
                        pv = ps_sc.tile([128, 512], BF, name="pst", tag="scr")
                        for m in range(4):
                            i = 4 * mg + m
                            nc.tensor.transpose(
                                pv[:, 128 * m : 128 * (m + 1)],
                                vT[h][:, 128 * i : 128 * (i + 1)],
                                ident_bf[:],
                            )
                        nc.scalar.activation(
                            vsb[:, 512 * mg : 512 * (mg + 1)], pv[:], AF.Copy
                        )

                    pv_ps = [
                        ps_pv.tile([128, 512], FP, name="pspv", tag="pspv")
                        for _ in range(NC)
                    ]
                    rs_ps = [
                        ps_rs.tile([128, 512], FP, name="psrs", tag="psrs")
                        for _ in range(2)
                    ]

                    for kb in range(NB):
                        k0 = 128 * kb
                        cmin = kb // 4
                        ept = ept_pool.tile([128, S], BF, name="ept", tag="ept")
                        # scores^T (k-major) + mask + exp, per 512-q chunk
                        for c in range(cmin, NC):
                            qlo = max(0, k0 - 512 * c)
                            q0 = 512 * c
                            scp = ps_sc.tile([128, 512], FP, name="pssc", tag="scr")
                            nc.tensor.matmul(
                                scp[:, qlo:512],
                                kT[h][:, k0 : k0 + 128],
                                qT[h][:, q0 + qlo : q0 + 512],
                                start=True,
                                stop=True,
                            )
                            if c == cmin:
                                nc.vector.tensor_add(
                                    scp[:, qlo : qlo + 128],
                                    scp[:, qlo : qlo + 128],
                                    maskT[:],
                                )
                            nc.scalar.activation(
                                ept[:, q0 + qlo : q0 + 512], scp[:, qlo:512], AF.Exp
                            )
                        # row sums (ones-column matmul), then PV, accumulated over kb
                        for c in range(cmin, NC):
                            qlo = max(0, k0 - 512 * c)
                            q0 = 512 * c
                            rt, rp = RS_POS[c]
                            nc.tensor.matmul(
                                rs_ps[rt][rp : rp + 1, qlo:512],
                                ones_col[:],
                                ept[:, q0 + qlo : q0 + 512],
                                start=(kb == 0),
                                stop=(kb == 4 * c + 3),
                            )
                        for c in range(cmin, NC):
                            qlo = max(0, k0 - 512 * c)
                            q0 = 512 * c
                            nc.tensor.matmul(
                                pv_ps[c][:, qlo:512],
                                vsb[:, k0 : k0 + 128],
                                ept[:, q0 + qlo : q0 + 512],
                                start=(kb == 0),
                                stop=(kb == 4 * c + 3),
                            )

                    # normalize: attO = pv / rowsum, reciprocal broadcast via PE
                    rs_sb = rsb_pool.tile([1, NC * 512], FP, name="rssb", tag="rssb")
                    for c in range(NC):
                        rt, rp = RS_POS[c]
                        nc.vector.reciprocal(
                            rs_sb[:, 512 * c : 512 * (c + 1)],
                            rs_ps[rt][rp : rp + 1, :],
                        )
                    for c in range(NC):
                        bc_ps = ps_sc.tile([128, 512], FP, name="psbc", tag="scr")
                        nc.tensor.matmul(
                            bc_ps[:],
                            ones_row[:],
                            rs_sb[:, 512 * c : 512 * (c + 1)],
                            start=True,
                            stop=True,
                        )
                        bc_sb = bcs_pool.tile([128, 512], FP, name="bcsb", tag="bcsb")
                        nc.scalar.activation(bc_sb[:], bc_ps[:], AF.Copy)
                        nc.vector.tensor_mul(
                            attO[h][:, 512 * c : 512 * (c + 1)],
                            pv_ps[c][:],
                            bc_sb[:],
                        )

        # ---------------- phase 3: output projection ----------------
        with ExitStack() as ph, nc.named_scope("outproj"):
            wo_pool = ph.enter_context(tc.tile_pool(name="wo", bufs=1))
            ostg = ph.enter_context(tc.tile_pool(name="ostg", bufs=4))
            ps_out = ph.enter_context(tc.tile_pool(name="ps_out", bufs=8, space="PSUM"))
            wos = []
            for h in range(H_LOC):
                wt = wo_pool.tile([128, E], BF, name=f"wo{h}", tag=f"wo{h}")
                nc.sync.dma_start(wt[:], wo[128 * h : 128 * (h + 1), :])
                wos.append(wt)
            nec = E // 512
            for i in range(NB):
                psums = [
                    ps_out.tile([128, 512], FP, name="pso", tag="pso")
                    for _ in range(nec)
                ]
                for h in range(H_LOC):
                    ah_blk = attO[h][:, 128 * i : 128 * (i + 1)]
                    for c in range(nec):
                        nc.tensor.matmul(
                            psums[c][:],
                            ah_blk,
                            wos[h][:, 512 * c : 512 * (c + 1)],
                            start=(h == 0),
                            stop=(h == H_LOC - 1),
                        )
                for c in range(nec):
                    ot = ostg.tile([128, 512], FP, name="ostg", tag="ostg")
                    nc.vector.tensor_copy(ot[:], psums[c][:])
                    nc.sync.dma_start(
                        out[128 * i : 128 * (i + 1), 512 * c : 512 * (c + 1)],
                        ot[:],
                    )


_NC_CACHE = None


def _get_nc():
    global _NC_CACHE
    if _NC_CACHE is None:
        nc = bacc.Bacc(
            "TRN2",
            target_bir_lowering=False,
            debug=False,
            num_devices=1,
            enable_asserts=False,
        )
        _emit(nc)
        nc.compile()
        _NC_CACHE = nc
    return _NC_CACHE


def make_in_maps(inX, W_qkv, b_qkv, W_out):
    bf = ml_dtypes.bfloat16
    xTs = [np.ascontiguousarray(inX[b].T.astype(bf)) for b in range(B)]
    in_maps = []
    for c in range(NCORES):
        b = c // HG
        hg = c % HG
        sl = slice(FLOC * hg, FLOC * (hg + 1))
        in_maps.append(
            {
                "xT": xTs[b],
                "wq": np.ascontiguousarray(
                    (W_qkv[:, 0:E][:, sl] * SCALE).astype(bf)
                ),
                "wk": np.ascontiguousarray(W_qkv[:, E : 2 * E][:, sl].astype(bf)),
                "wv": np.ascontiguousarray(W_qkv[:, 2 * E : 3 * E][:, sl].astype(bf)),
                "bqs": np.ascontiguousarray(
                    (b_qkv[0:E][sl] * SCALE).reshape(FLOC, 1).astype(np.float32)
                ),
                "bk": np.ascontiguousarray(
                    b_qkv[E : 2 * E][sl].reshape(FLOC, 1).astype(np.float32)
                ),
                "wo": np.ascontiguousarray(W_out[sl, :].astype(bf)),
            }
        )
    return in_maps


def kernel(inX, W_qkv, b_qkv, W_out, b_out):
    global LAST_EXEC_NS, LAST_RESULTS
    inX = np.asarray(inX, dtype=np.float32)
    W_qkv = np.asarray(W_qkv, dtype=np.float32)
    b_qkv = np.asarray(b_qkv, dtype=np.float32)
    W_out = np.asarray(W_out, dtype=np.float32)
    b_out = np.asarray(b_out, dtype=np.float32)

    nc = _get_nc()
    in_maps = make_in_maps(inX, W_qkv, b_qkv, W_out)

    kwargs = {}
    if PROFILE:
        kwargs = {"trace": True, "trace_cores": [0]}
    res = bass_utils.run_bass_kernel_spmd(
        nc, in_maps, core_ids=list(range(NCORES)), **kwargs
    )
    LAST_EXEC_NS = res.exec_time_ns
    LAST_RESULTS = res

    bias_full = (b_out + b_qkv[2 * E : 3 * E] @ W_out).astype(np.float32)
    out = np.empty((B, S, E), dtype=np.float32)
    for b in range(B):
        acc = res.results[HG * b + 0]["out"].astype(np.float64)
        for hg in range(1, HG):
            acc += res.results[HG * b + hg]["out"]
        out[b] = (acc + bias_full).astype(np.float32)
    return out


# revision 15
# speedup vs baseline: 2.3083x; 1.0946x over previous
"""Causal self-attention (B=2, S=2048, E=2048, H=16) on 8 TRN2 NeuronCores.

Sharding: 2-way batch x 4-way head-group tensor parallel.
Core c handles batch c//4 and heads [4*(c%4), 4*(c%4)+4).

Single-pass bf16 design (the rel-err gate is 2e-2; bf16 lands ~3e-3):
  phase 1: QKV projection from host-pretransposed X^T (bf16), feature-major
           qT/kT/vT [128hd, S] bf16 kept in SBUF (no DRAM staging).
           SCALE is folded into Wq/bq on the host.
  phase 2: per head: k-major scores (stationary kT block, moving qT) ->
           transposed-causal mask -> exp to bf16 expPT (k-major, which is
           exactly the PV moving layout: no P transposes). Row sums via a
           ones-column matmul accumulated alongside PV; normalization is
           applied to the (small) PV output, with the per-q reciprocal
           broadcast across partitions by a tiny K=1 matmul.
  phase 3: out projection: stationary attO blocks (feature-major), moving
           W_out rows; fp32 partials to DRAM.

Host side: per batch X^T in bf16 (shared by 4 cores), per head-group W
slices in bf16; sum the 4 head-group partials per batch and add
(b_out + b_v @ W_out) once (softmax rows sum to 1, so the v-bias
contribution is a constant row vector).
"""

from contextlib import ExitStack

import ml_dtypes
import numpy as np

import concourse.bass as bass
import concourse.tile as tile
from concourse import bacc, bass_utils, mybir
from concourse.masks import make_identity

FP = mybir.dt.float32
BF = mybir.dt.bfloat16
AF = mybir.ActivationFunctionType

B = 2
S = 2048
E = 2048
H = 16
HD = 128
NCORES = 8
HG = 4  # head-group axis (tensor parallel)
H_LOC = H // HG  # 4 heads per core
FLOC = H_LOC * HD  # 512 local features per q/k/v
SCALE = 1.0 / float(np.sqrt(HD))
NEG = -1.0e30

PROFILE = False
LAST_EXEC_NS = None
LAST_RESULTS = None


def _emit(nc, S=S, E=E):
    NB = S // 128  # token blocks
    EB = E // 128  # embed blocks
    NC = S // 512  # 512-wide q chunks

    xT = nc.dram_tensor("xT", [E, S], BF, kind="ExternalInput").ap()
    wq = nc.dram_tensor("wq", [E, FLOC], BF, kind="ExternalInput").ap()
    wk = nc.dram_tensor("wk", [E, FLOC], BF, kind="ExternalInput").ap()
    wv = nc.dram_tensor("wv", [E, FLOC], BF, kind="ExternalInput").ap()
    bqs = nc.dram_tensor("bqs", [FLOC, 1], FP, kind="ExternalInput").ap()  # *SCALE
    bk = nc.dram_tensor("bk", [FLOC, 1], FP, kind="ExternalInput").ap()
    wo = nc.dram_tensor("wo", [FLOC, E], BF, kind="ExternalInput").ap()
    out = nc.dram_tensor("out", [S, E], FP, kind="ExternalOutput").ap()

    with tile.TileContext(nc) as tc, ExitStack() as top:
        cst = top.enter_context(tc.tile_pool(name="cst", bufs=1))
        ident_bf = cst.tile([128, 128], BF, name="identbf", tag="identbf")
        make_identity(nc, ident_bf[:])
        # transposed causal mask: keep (0) where q(free) >= k(part), NEG below
        maskT = cst.tile([128, 128], FP, name="maskT", tag="maskT")
        nc.gpsimd.memset(maskT[:], 0.0)
        nc.gpsimd.affine_select(
            out=maskT[:],
            in_=maskT[:],
            compare_op=mybir.AluOpType.is_ge,
            fill=NEG,
            base=0,
            pattern=[[1, 128]],  # +1 * free index
            channel_multiplier=-1,  # -1 * partition index
        )
        ones_col = cst.tile([128, 1], BF, name="onescol", tag="onescol")
        nc.vector.memset(ones_col[:], 1.0)
        ones_row = cst.tile([1, 128], FP, name="onesrow", tag="onesrow")
        nc.vector.memset(ones_row[:], 1.0)
        bq_sb = cst.tile([128, H_LOC], FP, name="bq", tag="bq")
        bk_sb = cst.tile([128, H_LOC], FP, name="bk", tag="bk")
        for f in range(H_LOC):
            nc.sync.dma_start(bq_sb[:, f : f + 1], bqs[128 * f : 128 * (f + 1), :])
            nc.sync.dma_start(bk_sb[:, f : f + 1], bk[128 * f : 128 * (f + 1), :])

        # feature-major q/k/v, bf16, SBUF-resident
        qkv_pool = top.enter_context(tc.tile_pool(name="qkvT", bufs=1))
        qT = [qkv_pool.tile([128, S], BF, name=f"qT{h}", tag=f"qT{h}") for h in range(H_LOC)]
        kT = [qkv_pool.tile([128, S], BF, name=f"kT{h}", tag=f"kT{h}") for h in range(H_LOC)]
        vT = [qkv_pool.tile([128, S], BF, name=f"vT{h}", tag=f"vT{h}") for h in range(H_LOC)]
        qkvT = [qT, kT, vT]
        # feature-major attention outputs, bf16, SBUF-resident
        att_pool = top.enter_context(tc.tile_pool(name="att", bufs=1))
        attO = [
            att_pool.tile([128, S], BF, name=f"attO{h}", tag=f"attO{h}")
            for h in range(H_LOC)
        ]

        # ---------------- phase 1: QKV projection ----------------
        with ExitStack() as ph, nc.named_scope("proj"):
            xt_pool = ph.enter_context(tc.tile_pool(name="xt", bufs=1))
            xts = []
            for e in range(EB):
                t = xt_pool.tile([128, S], BF, name=f"xt{e}", tag=f"xt{e}")
                nc.sync.dma_start(t[:], xT[128 * e : 128 * (e + 1), :])
                xts.append(t)
            wpool = ph.enter_context(tc.tile_pool(name="w", bufs=1))
            ps_qkv = ph.enter_context(tc.tile_pool(name="ps_qkv", bufs=8, space="PSUM"))
            # one DMA per (projection, e-block): [128, FLOC] slabs
            wsb = []
            for wi, wsrc in enumerate((wq, wk, wv)):
                t = wpool.tile([128, EB * FLOC], BF, name=f"wsb{wi}", tag=f"wsb{wi}")
                for e in range(EB):
                    nc.sync.dma_start(
                        t[:, FLOC * e : FLOC * (e + 1)],
                        wsrc[128 * e : 128 * (e + 1), :],
                    )
                wsb.append(t)

            for which in range(3):
                for h in range(H_LOC):
                    psums = [
                        ps_qkv.tile([128, 512], FP, name="psq", tag="psq")
                        for _ in range(NC)
                    ]
                    for e in range(EB):
                        wt = wsb[which][
                            :, FLOC * e + 128 * h : FLOC * e + 128 * (h + 1)
                        ]
                        first = e == 0
                        last = e == EB - 1
                        for sc in range(NC):
                            nc.tensor.matmul(
                                psums[sc][:],
                                wt,
                                xts[e][:, 512 * sc : 512 * (sc + 1)],
                                start=first,
                                stop=last,
                            )
                    dst = qkvT[which][h]
                    for sc in range(NC):
                        sl = slice(512 * sc, 512 * (sc + 1))
                        if which == 0:
                            nc.vector.tensor_scalar_add(
                                dst[:, sl], psums[sc][:], bq_sb[:, h : h + 1]
                            )
                        elif which == 1:
                            nc.vector.tensor_scalar_add(
                                dst[:, sl], psums[sc][:], bk_sb[:, h : h + 1]
                            )
                        else:
                            nc.scalar.activation(dst[:, sl], psums[sc][:], AF.Copy)

        # ---------------- phase 2: attention per head ----------------
        with ExitStack() as ah:
            vsb_pool = ah.enter_context(tc.tile_pool(name="vsb", bufs=2))
            ept_pool = ah.enter_context(tc.tile_pool(name="ept", bufs=2))
            rsb_pool = ah.enter_context(tc.tile_pool(name="rsb", bufs=2))
            bcs_pool = ah.enter_context(tc.tile_pool(name="bcs", bufs=2))
            ps_sc = ah.enter_context(tc.tile_pool(name="ps_sc", bufs=2, space="PSUM"))
            ps_pv = ah.enter_context(tc.tile_pool(name="ps_pv", bufs=4, space="PSUM"))
            ps_rs = ah.enter_context(tc.tile_pool(name="ps_rs", bufs=2, space="PSUM"))
            # PE matmul outputs must start at partition 0/32/64: spread the
            # 4 per-chunk rowsum rows over two banks at those offsets
            RS_POS = [(0, 0), (0, 32), (0, 64), (1, 0)]

            for h in range(H_LOC):
                with nc.named_scope(f"attn{h}"):
                    # V -> token-major [128 tok, hd] blocks along S
                    vsb = vsb_pool.tile([128, S], BF, name="vsb", tag="vsb")
                    for mg in range(NB // 4):
                        pv = ps_sc.tile([128, 512], BF, name="pst", tag="scr")
                        for m in range(4):
                            i = 4 * mg + m
                            nc.tensor.transpose(
                                pv[:, 128 * m : 128 * (m + 1)],
                                vT[h][:, 128 * i : 128 * (i + 1)],
                                ident_bf[:],
                            )
                        nc.scalar.activation(
                            vsb[:, 512 * mg : 512 * (mg + 1)], pv[:], AF.Copy
                        )

                    pv_ps = [
                        ps_pv.tile([128, 512], FP, name="pspv", tag="pspv")
                        for _ in range(NC)
                    ]
                    rs_ps = [
                        ps_rs.tile([128, 512], FP, name="psrs", tag="psrs")
                        for _ in range(2)
                    ]

                    for kb in range(NB):
                        k0 = 128 * kb
                        cmin = kb // 4
                        ept = ept_pool.tile([128, S], BF, name="ept", tag="ept")
                        # scores^T (k-major) + mask + exp, per 512-q chunk
                        for c in range(cmin, NC):
                            qlo = max(0, k0 - 512 * c)
                            q0 = 512 * c
                            scp = ps_sc.tile([128, 512], FP, name="pssc", tag="scr")
                            nc.tensor.matmul(
                                scp[:, qlo:512],
                                kT[h][:, k0 : k0 + 128],
                                qT[h][:, q0 + qlo : q0 + 512],
                                start=True,
                                stop=True,
                            )
                            if c == cmin:
                                nc.vector.tensor_add(
                                    scp[:, qlo : qlo + 128],
                                    scp[:, qlo : qlo + 128],
                                    maskT[:],
                                )
                            nc.scalar.activation(
                                ept[:, q0 + qlo : q0 + 512], scp[:, qlo:512], AF.Exp
                            )
                        # row sums (ones-column matmul), then PV, accumulated over kb
                        for c in range(cmin, NC):
                            qlo = max(0, k0 - 512 * c)
                            q0 = 512 * c
                            rt, rp = RS_POS[c]
                            nc.tensor.matmul(
                                rs_ps[rt][rp : rp + 1, qlo:512],
                                ones_col[:],
                                ept[:, q0 + qlo : q0 + 512],
                                start=(kb == 0),
                                stop=(kb == 4 * c + 3),
                            )
                        for c in range(cmin, NC):
                            qlo = max(0, k0 - 512 * c)
                            q0 = 512 * c
                            nc.tensor.matmul(
                                pv_ps[c][:, qlo:512],
                                vsb[:, k0 : k0 + 128],
                                ept[:, q0 + qlo : q0 + 512],
                                start=(kb == 0),
                                stop=(kb == 4 * c + 3),
                            )

                    # normalize: attO = pv / rowsum, reciprocal broadcast via PE
                    rs_sb = rsb_pool.tile([1, NC * 512], FP, name="rssb", tag="rssb")
                    for c in range(NC):
                        rt, rp = RS_POS[c]
                        nc.vector.reciprocal(
                            rs_sb[:, 512 * c : 512 * (c + 1)],
                            rs_ps[rt][rp : rp + 1, :],
                        )
                    for c in range(NC):
                        bc_ps = ps_sc.tile([128, 512], FP, name="psbc", tag="scr")
                        nc.tensor.matmul(
                            bc_ps[:],
                            ones_row[:],
                            rs_sb[:, 512 * c : 512 * (c + 1)],
                            start=True,
                            stop=True,
                        )
                        bc_sb = bcs_pool.tile([128, 512], FP, name="bcsb", tag="bcsb")
                        nc.scalar.activation(bc_sb[:], bc_ps[:], AF.Copy)
                        nc.vector.tensor_mul(
                            attO[h][:, 512 * c : 512 * (c + 1)],
                            pv_ps[c][:],
                            bc_sb[:],
                        )

        # ---------------- phase 3: output projection ----------------
        with ExitStack() as ph, nc.named_scope("outproj"):
            wo_pool = ph.enter_context(tc.tile_pool(name="wo", bufs=1))
            ostg = ph.enter_context(tc.tile_pool(name="ostg", bufs=4))
            ps_out = ph.enter_context(tc.tile_pool(name="ps_out", bufs=8, space="PSUM"))
            wos = []
            for h in range(H_LOC):
                wt = wo_pool.tile([128, E], BF, name=f"wo{h}", tag=f"wo{h}")
                nc.sync.dma_start(wt[:], wo[128 * h : 128 * (h + 1), :])
                wos.append(wt)
            nec = E // 512
            for i in range(NB):
                psums = [
                    ps_out.tile([128, 512], FP, name="pso", tag="pso")
                    for _ in range(nec)
                ]
                for h in range(H_LOC):
                    ah_blk = attO[h][:, 128 * i : 128 * (i + 1)]
                    for c in range(nec):
                        nc.tensor.matmul(
                            psums[c][:],
                            ah_blk,
                            wos[h][:, 512 * c : 512 * (c + 1)],
                            start=(h == 0),
                            stop=(h == H_LOC - 1),
                        )
                for c in range(nec):
                    ot = ostg.tile([128, 512], FP, name="ostg", tag="ostg")
                    if c % 2 == 0:
                        nc.vector.tensor_copy(ot[:], psums[c][:])
                    else:
                        nc.scalar.activation(ot[:], psums[c][:], AF.Copy)
                    nc.sync.dma_start(
                        out[128 * i : 128 * (i + 1), 512 * c : 512 * (c + 1)],
                        ot[:],
                    )


_NC_CACHE = None


def _get_nc():
    global _NC_CACHE
    if _NC_CACHE is None:
        nc = bacc.Bacc(
            "TRN2",
            target_bir_lowering=False,
            debug=False,
            num_devices=1,
            enable_asserts=False,
        )
        _emit(nc)
        nc.compile()
        _NC_CACHE = nc
    return _NC_CACHE


def make_in_maps(inX, W_qkv, b_qkv, W_out):
    bf = ml_dtypes.bfloat16
    xTs = [np.ascontiguousarray(inX[b].T.astype(bf)) for b in range(B)]
    in_maps = []
    for c in range(NCORES):
        b = c // HG
        hg = c % HG
        sl = slice(FLOC * hg, FLOC * (hg + 1))
        in_maps.append(
            {
                "xT": xTs[b],
                "wq": np.ascontiguousarray(
                    (W_qkv[:, 0:E][:, sl] * SCALE).astype(bf)
                ),
                "wk": np.ascontiguousarray(W_qkv[:, E : 2 * E][:, sl].astype(bf)),
                "wv": np.ascontiguousarray(W_qkv[:, 2 * E : 3 * E][:, sl].astype(bf)),
                "bqs": np.ascontiguousarray(
                    (b_qkv[0:E][sl] * SCALE).reshape(FLOC, 1).astype(np.float32)
                ),
                "bk": np.ascontiguousarray(
                    b_qkv[E : 2 * E][sl].reshape(FLOC, 1).astype(np.float32)
                ),
                "wo": np.ascontiguousarray(W_out[sl, :].astype(bf)),
            }
        )
    return in_maps


def kernel(inX, W_qkv, b_qkv, W_out, b_out):
    global LAST_EXEC_NS, LAST_RESULTS
    inX = np.asarray(inX, dtype=np.float32)
    W_qkv = np.asarray(W_qkv, dtype=np.float32)
    b_qkv = np.asarray(b_qkv, dtype=np.float32)
    W_out = np.asarray(W_out, dtype=np.float32)
    b_out = np.asarray(b_out, dtype=np.float32)

    nc = _get_nc()
    in_maps = make_in_maps(inX, W_qkv, b_qkv, W_out)

    kwargs = {}
    if PROFILE:
        kwargs = {"trace": True, "trace_cores": [0]}
    res = bass_utils.run_bass_kernel_spmd(
        nc, in_maps, core_ids=list(range(NCORES)), **kwargs
    )
    LAST_EXEC_NS = res.exec_time_ns
    LAST_RESULTS = res

    bias_full = (b_out + b_qkv[2 * E : 3 * E] @ W_out).astype(np.float32)
    out = np.empty((B, S, E), dtype=np.float32)
    for b in range(B):
        acc = res.results[HG * b + 0]["out"].astype(np.float64)
        for hg in range(1, HG):
            acc += res.results[HG * b + hg]["out"]
        out[b] = (acc + bias_full).astype(np.float32)
    return out


# revision 18
# speedup vs baseline: 2.4550x; 1.0635x over previous
"""Causal self-attention (B=2, S=2048, E=2048, H=16) on 8 TRN2 NeuronCores.

Sharding: 2-way batch x 4-way head-group tensor parallel.
Core c handles batch c//4 and heads [4*(c%4), 4*(c%4)+4).

Single-pass bf16 design (the rel-err gate is 2e-2; bf16 lands ~3e-3):
  phase 1: QKV projection from host-pretransposed X^T (bf16), feature-major
           qT/kT/vT [128hd, S] bf16 kept in SBUF (no DRAM staging).
           SCALE is folded into Wq/bq on the host.
  phase 2: per head: k-major scores (stationary kT block, moving qT) ->
           transposed-causal mask -> exp to bf16 expPT (k-major, which is
           exactly the PV moving layout: no P transposes). Row sums via a
           ones-column matmul accumulated alongside PV; normalization is
           applied to the (small) PV output, with the per-q reciprocal
           broadcast across partitions by a tiny K=1 matmul.
  phase 3: out projection: stationary attO blocks (feature-major), moving
           W_out rows; fp32 partials to DRAM.

Host side: per batch X^T in bf16 (shared by 4 cores), per head-group W
slices in bf16; sum the 4 head-group partials per batch and add
(b_out + b_v @ W_out) once (softmax rows sum to 1, so the v-bias
contribution is a constant row vector).
"""

from contextlib import ExitStack

import ml_dtypes
import numpy as np

import concourse.bass as bass
import concourse.tile as tile
from concourse import bacc, bass_utils, mybir
from concourse.masks import make_identity

FP = mybir.dt.float32
BF = mybir.dt.bfloat16
AF = mybir.ActivationFunctionType

B = 2
S = 2048
E = 2048
H = 16
HD = 128
NCORES = 8
HG = 4  # head-group axis (tensor parallel)
H_LOC = H // HG  # 4 heads per core
FLOC = H_LOC * HD  # 512 local features per q/k/v
SCALE = 1.0 / float(np.sqrt(HD))
NEG = -1.0e30

PROFILE = False
LAST_EXEC_NS = None
LAST_RESULTS = None


def _emit(nc, S=S, E=E):
    NB = S // 128  # token blocks
    EB = E // 128  # embed blocks
    NC = S // 512  # 512-wide q chunks

    xT = nc.dram_tensor("xT", [E, S], BF, kind="ExternalInput").ap()
    wq = nc.dram_tensor("wq", [E, FLOC], BF, kind="ExternalInput").ap()
    wk = nc.dram_tensor("wk", [E, FLOC], BF, kind="ExternalInput").ap()
    wv = nc.dram_tensor("wv", [E, FLOC], BF, kind="ExternalInput").ap()
    bqs = nc.dram_tensor("bqs", [FLOC, 1], FP, kind="ExternalInput").ap()  # *SCALE
    bk = nc.dram_tensor("bk", [FLOC, 1], FP, kind="ExternalInput").ap()
    wo = nc.dram_tensor("wo", [FLOC, E], BF, kind="ExternalInput").ap()
    out = nc.dram_tensor("out", [S, E], FP, kind="ExternalOutput").ap()

    with tile.TileContext(nc) as tc, ExitStack() as top:
        cst = top.enter_context(tc.tile_pool(name="cst", bufs=1))
        ident_bf = cst.tile([128, 128], BF, name="identbf", tag="identbf")
        make_identity(nc, ident_bf[:])
        # transposed causal mask: keep (0) where q(free) >= k(part), NEG below
        maskT = cst.tile([128, 128], FP, name="maskT", tag="maskT")
        nc.gpsimd.memset(maskT[:], 0.0)
        nc.gpsimd.affine_select(
            out=maskT[:],
            in_=maskT[:],
            compare_op=mybir.AluOpType.is_ge,
            fill=NEG,
            base=0,
            pattern=[[1, 128]],  # +1 * free index
            channel_multiplier=-1,  # -1 * partition index
        )
        ones_col = cst.tile([128, 1], BF, name="onescol", tag="onescol")
        nc.vector.memset(ones_col[:], 1.0)
        ones_row = cst.tile([1, 128], FP, name="onesrow", tag="onesrow")
        nc.vector.memset(ones_row[:], 1.0)
        bq_sb = cst.tile([128, H_LOC], FP, name="bq", tag="bq")
        bk_sb = cst.tile([128, H_LOC], FP, name="bk", tag="bk")
        for f in range(H_LOC):
            nc.sync.dma_start(bq_sb[:, f : f + 1], bqs[128 * f : 128 * (f + 1), :])
            nc.sync.dma_start(bk_sb[:, f : f + 1], bk[128 * f : 128 * (f + 1), :])

        # feature-major q/k/v, bf16, SBUF-resident
        qkv_pool = top.enter_context(tc.tile_pool(name="qkvT", bufs=1))
        qT = [qkv_pool.tile([128, S], BF, name=f"qT{h}", tag=f"qT{h}") for h in range(H_LOC)]
        kT = [qkv_pool.tile([128, S], BF, name=f"kT{h}", tag=f"kT{h}") for h in range(H_LOC)]
        vT = [qkv_pool.tile([128, S], BF, name=f"vT{h}", tag=f"vT{h}") for h in range(H_LOC)]
        qkvT = [qT, kT, vT]
        # feature-major attention outputs, bf16, SBUF-resident
        att_pool = top.enter_context(tc.tile_pool(name="att", bufs=1))
        attO = [
            att_pool.tile([128, S], BF, name=f"attO{h}", tag=f"attO{h}")
            for h in range(H_LOC)
        ]

        # ---------------- phase 1: QKV projection ----------------
        with ExitStack() as ph, nc.named_scope("proj"):
            xt_pool = ph.enter_context(tc.tile_pool(name="xt", bufs=1))
            wpool = ph.enter_context(tc.tile_pool(name="w", bufs=1))
            ps_qkv = ph.enter_context(tc.tile_pool(name="ps_qkv", bufs=8, space="PSUM"))
            xts = [
                xt_pool.tile([128, S], BF, name=f"xt{e}", tag=f"xt{e}")
                for e in range(EB)
            ]
            wsb = [
                wpool.tile([128, EB * FLOC], BF, name=f"wsb{wi}", tag=f"wsb{wi}")
                for wi in range(3)
            ]
            # DMA order = first-use order: wq/xt slabs interleaved (the first
            # accumulation chain consumes them e-ascending), then wk, wv
            for e in range(EB):
                nc.sync.dma_start(
                    wsb[0][:, FLOC * e : FLOC * (e + 1)],
                    wq[128 * e : 128 * (e + 1), :],
                )
                nc.sync.dma_start(xts[e][:], xT[128 * e : 128 * (e + 1), :])
            for wi, wsrc in ((1, wk), (2, wv)):
                for e in range(EB):
                    nc.sync.dma_start(
                        wsb[wi][:, FLOC * e : FLOC * (e + 1)],
                        wsrc[128 * e : 128 * (e + 1), :],
                    )

            for which in range(3):
                for h in range(H_LOC):
                    psums = [
                        ps_qkv.tile([128, 512], FP, name="psq", tag="psq")
                        for _ in range(NC)
                    ]
                    for e in range(EB):
                        wt = wsb[which][
                            :, FLOC * e + 128 * h : FLOC * e + 128 * (h + 1)
                        ]
                        first = e == 0
                        last = e == EB - 1
                        for sc in range(NC):
                            nc.tensor.matmul(
                                psums[sc][:],
                                wt,
                                xts[e][:, 512 * sc : 512 * (sc + 1)],
                                start=first,
                                stop=last,
                            )
                    dst = qkvT[which][h]
                    for sc in range(NC):
                        sl = slice(512 * sc, 512 * (sc + 1))
                        if which == 0:
                            nc.vector.tensor_scalar_add(
                                dst[:, sl], psums[sc][:], bq_sb[:, h : h + 1]
                            )
                        elif which == 1:
                            nc.vector.tensor_scalar_add(
                                dst[:, sl], psums[sc][:], bk_sb[:, h : h + 1]
                            )
                        else:
                            nc.scalar.activation(dst[:, sl], psums[sc][:], AF.Copy)

        # ---------------- phase 2: attention per head ----------------
        with ExitStack() as ah:
            vsb_pool = ah.enter_context(tc.tile_pool(name="vsb", bufs=2))
            ept_pool = ah.enter_context(tc.tile_pool(name="ept", bufs=2))
            rsb_pool = ah.enter_context(tc.tile_pool(name="rsb", bufs=2))
            bcs_pool = ah.enter_context(tc.tile_pool(name="bcs", bufs=2))
            ps_sc = ah.enter_context(tc.tile_pool(name="ps_sc", bufs=2, space="PSUM"))
            ps_pv = ah.enter_context(tc.tile_pool(name="ps_pv", bufs=4, space="PSUM"))
            ps_rs = ah.enter_context(tc.tile_pool(name="ps_rs", bufs=2, space="PSUM"))
            # PE matmul outputs must start at partition 0/32/64: spread the
            # 4 per-chunk rowsum rows over two banks at those offsets
            RS_POS = [(0, 0), (0, 32), (0, 64), (1, 0)]

            for h in range(H_LOC):
                with nc.named_scope(f"attn{h}"):
                    # V -> token-major [128 tok, hd] blocks along S
                    vsb = vsb_pool.tile([128, S], BF, name="vsb", tag="vsb")
                    for mg in range(NB // 4):
                        pv = ps_sc.tile([128, 512], BF, name="pst", tag="scr")
                        for m in range(4):
                            i = 4 * mg + m
                            nc.tensor.transpose(
                                pv[:, 128 * m : 128 * (m + 1)],
                                vT[h][:, 128 * i : 128 * (i + 1)],
                                ident_bf[:],
                            )
                        nc.scalar.activation(
                            vsb[:, 512 * mg : 512 * (mg + 1)], pv[:], AF.Copy
                        )

                    pv_ps = [
                        ps_pv.tile([128, 512], FP, name="pspv", tag="pspv")
                        for _ in range(NC)
                    ]
                    rs_ps = [
                        ps_rs.tile([128, 512], FP, name="psrs", tag="psrs")
                        for _ in range(2)
                    ]

                    # software pipeline: rs/PV for kb-1 run while exp(kb) is
                    # still on the scalar engine, so the PE never waits on exp
                    epts = [None] * NB
                    for kb in range(NB + 1):
                        if kb < NB:
                            k0 = 128 * kb
                            cmin = kb // 4
                            ept = ept_pool.tile([128, S], BF, name="ept", tag="ept")
                            epts[kb] = ept
                            # scores^T (k-major) + mask + exp, per 512-q chunk
                            for c in range(cmin, NC):
                                qlo = max(0, k0 - 512 * c)
                                q0 = 512 * c
                                scp = ps_sc.tile(
                                    [128, 512], FP, name="pssc", tag="scr"
                                )
                                nc.tensor.matmul(
                                    scp[:, qlo:512],
                                    kT[h][:, k0 : k0 + 128],
                                    qT[h][:, q0 + qlo : q0 + 512],
                                    start=True,
                                    stop=True,
                                )
                                if c == cmin:
                                    nc.vector.tensor_add(
                                        scp[:, qlo : qlo + 128],
                                        scp[:, qlo : qlo + 128],
                                        maskT[:],
                                    )
                                nc.scalar.activation(
                                    ept[:, q0 + qlo : q0 + 512],
                                    scp[:, qlo:512],
                                    AF.Exp,
                                )
                        if kb == 0:
                            continue
                        pb = kb - 1
                        k0 = 128 * pb
                        cmin = pb // 4
                        ept = epts[pb]
                        # row sums (ones-column matmul), then PV
                        for c in range(cmin, NC):
                            qlo = max(0, k0 - 512 * c)
                            q0 = 512 * c
                            rt, rp = RS_POS[c]
                            nc.tensor.matmul(
                                rs_ps[rt][rp : rp + 1, qlo:512],
                                ones_col[:],
                                ept[:, q0 + qlo : q0 + 512],
                                start=(pb == 0),
                                stop=(pb == 4 * c + 3),
                            )
                        for c in range(cmin, NC):
                            qlo = max(0, k0 - 512 * c)
                            q0 = 512 * c
                            nc.tensor.matmul(
                                pv_ps[c][:, qlo:512],
                                vsb[:, k0 : k0 + 128],
                                ept[:, q0 + qlo : q0 + 512],
                                start=(pb == 0),
                                stop=(pb == 4 * c + 3),
                            )

                    # normalize: attO = pv / rowsum, reciprocal broadcast via PE
                    rs_sb = rsb_pool.tile([1, NC * 512], FP, name="rssb", tag="rssb")
                    for c in range(NC):
                        rt, rp = RS_POS[c]
                        nc.vector.reciprocal(
                            rs_sb[:, 512 * c : 512 * (c + 1)],
                            rs_ps[rt][rp : rp + 1, :],
                        )
                    for c in range(NC):
                        bc_ps = ps_rs.tile([128, 512], FP, name="psbc", tag="psrs")
                        nc.tensor.matmul(
                            bc_ps[:],
                            ones_row[:],
                            rs_sb[:, 512 * c : 512 * (c + 1)],
                            start=True,
                            stop=True,
                        )
                        bc_sb = bcs_pool.tile([128, 512], FP, name="bcsb", tag="bcsb")
                        nc.scalar.activation(bc_sb[:], bc_ps[:], AF.Copy)
                        nc.vector.tensor_mul(
                            attO[h][:, 512 * c : 512 * (c + 1)],
                            pv_ps[c][:],
                            bc_sb[:],
                        )

        # ---------------- phase 3: output projection ----------------
        with ExitStack() as ph, nc.named_scope("outproj"):
            wo_pool = ph.enter_context(tc.tile_pool(name="wo", bufs=1))
            ostg = ph.enter_context(tc.tile_pool(name="ostg", bufs=4))
            ps_out = ph.enter_context(tc.tile_pool(name="ps_out", bufs=8, space="PSUM"))
            wos = []
            for h in range(H_LOC):
                wt = wo_pool.tile([128, E], BF, name=f"wo{h}", tag=f"wo{h}")
                nc.sync.dma_start(wt[:], wo[128 * h : 128 * (h + 1), :])
                wos.append(wt)
            nec = E // 512
            for i in range(NB):
                psums = [
                    ps_out.tile([128, 512], FP, name="pso", tag="pso")
                    for _ in range(nec)
                ]
                for h in range(H_LOC):
                    ah_blk = attO[h][:, 128 * i : 128 * (i + 1)]
                    for c in range(nec):
                        nc.tensor.matmul(
                            psums[c][:],
                            ah_blk,
                            wos[h][:, 512 * c : 512 * (c + 1)],
                            start=(h == 0),
                            stop=(h == H_LOC - 1),
                        )
                for c in range(nec):
                    ot = ostg.tile([128, 512], FP, name="ostg", tag="ostg")
                    if c % 2 == 0:
                        nc.vector.tensor_copy(ot[:], psums[c][:])
                    else:
                        nc.scalar.activation(ot[:], psums[c][:], AF.Copy)
                    nc.sync.dma_start(
                        out[128 * i : 128 * (i + 1), 512 * c : 512 * (c + 1)],
                        ot[:],
                    )


_NC_CACHE = None


def _get_nc():
    global _NC_CACHE
    if _NC_CACHE is None:
        nc = bacc.Bacc(
            "TRN2",
            target_bir_lowering=False,
            debug=False,
            num_devices=1,
            enable_asserts=False,
        )
        _emit(nc)
        nc.compile()
        _NC_CACHE = nc
    return _NC_CACHE


def make_in_maps(inX, W_qkv, b_qkv, W_out):
    bf = ml_dtypes.bfloat16
    xTs = [np.ascontiguousarray(inX[b].T.astype(bf)) for b in range(B)]
    in_maps = []
    for c in range(NCORES):
        b = c // HG
        hg = c % HG
        sl = slice(FLOC * hg, FLOC * (hg + 1))
        in_maps.append(
            {
                "xT": xTs[b],
                "wq": np.ascontiguousarray(
                    (W_qkv[:, 0:E][:, sl] * SCALE).astype(bf)
                ),
                "wk": np.ascontiguousarray(W_qkv[:, E : 2 * E][:, sl].astype(bf)),
                "wv": np.ascontiguousarray(W_qkv[:, 2 * E : 3 * E][:, sl].astype(bf)),
                "bqs": np.ascontiguousarray(
                    (b_qkv[0:E][sl] * SCALE).reshape(FLOC, 1).astype(np.float32)
                ),
                "bk": np.ascontiguousarray(
                    b_qkv[E : 2 * E][sl].reshape(FLOC, 1).astype(np.float32)
                ),
                "wo": np.ascontiguousarray(W_out[sl, :].astype(bf)),
            }
        )
    return in_maps


def kernel(inX, W_qkv, b_qkv, W_out, b_out):
    global LAST_EXEC_NS, LAST_RESULTS
    inX = np.asarray(inX, dtype=np.float32)
    W_qkv = np.asarray(W_qkv, dtype=np.float32)
    b_qkv = np.asarray(b_qkv, dtype=np.float32)
    W_out = np.asarray(W_out, dtype=np.float32)
    b_out = np.asarray(b_out, dtype=np.float32)

    nc = _get_nc()
    in_maps = make_in_maps(inX, W_qkv, b_qkv, W_out)

    kwargs = {}
    if PROFILE:
        kwargs = {"trace": True, "trace_cores": [0]}
    res = bass_utils.run_bass_kernel_spmd(
        nc, in_maps, core_ids=list(range(NCORES)), **kwargs
    )
    LAST_EXEC_NS = res.exec_time_ns
    LAST_RESULTS = res

    bias_full = (b_out + b_qkv[2 * E : 3 * E] @ W_out).astype(np.float32)
    out = np.empty((B, S, E), dtype=np.float32)
    for b in range(B):
        acc = res.results[HG * b + 0]["out"].astype(np.float64)
        for hg in range(1, HG):
            acc += res.results[HG * b + hg]["out"]
        out[b] = (acc + bias_full).astype(np.float32)
    return out
